# revision 34
# baseline (speedup 1.0000x reference)
"""LSTMCell on 8 Trainium2 NeuronCores, data-parallel over the batch.

Full inputs: x/h_t/c_t [65536,128] f32, 8 gate weight matrices [128,128],
4 biases [128]. Returns (h_new, c_new) as [65536,128] f32 each.

Design (v13, ~59.7us; fp16 matmul path, transposed layout, no on-device
transposes; steady state is ACT(sigmoid)-bound):
  - Host transposes x/h/c per core to [128 feat, 8192 batch] fp16 and
    pre-concats weights as WxT/WhT [128 in, 512 gate-rows] fp16 in gate
    order [o, i, f, 2*g] (g prescaled by 2 for the tanh-via-sigmoid trick;
    o first so the first/last pairs can sigmoid banks i|f|2g ahead of o,
    unblocking the DVE chain ~1us earlier at the fill and tail).
    fp16 (not bf16) operands: the bf16 rounding of x/h/W through the gates
    was the dominant error term (1.2e-2); fp16 cuts it ~8x at zero cost
    (PE streams fp16 == bf16: ~216-260ns issue period per N=512 matmul).
  - Per batch group of 512 cols: 8 matmuls (weights stationary) accumulate
    gates^T into a 4-bank PSUM quad [128, 2048] = o|i|f|2g pre-acts.
  - ONE sigmoid per quad -> bf16 SBUF (ACT 16-bit-out runs ~0.9ns/elem;
    bf16 out is fastest of the 16-bit options; f32-out would be 2x faster
    on ACT but forces the DVE chain to 1x mode = net loss). Two groups
    share a sig2 tile [128, 4096] so DVE ops batch per PAIR via 3D APs
    (2-byte dtypes keep the DVE 2x mode, ~0.67ns/elem).
  - DVE per pair: gt=2s-1 [TS], ig=i*gt, fc=f*c, c'=ig+fc, h'=o*tanh(c').
    ig/fc/c'/h' and the c input are fp16: bf16 rounding of the large ig/fc
    terms dominated the error after cancellation in c'.
  - tanh(c') on ACT, delayed one pair (emitted after the next pair's
    sigmoids) so ACT never stalls on the DVE chain; batched across 2 pairs
    mid-pipeline; per-group at the tail to shorten the critical path.
  - DMA: inputs in 5 chunks/tensor (1,1,2,4,8 groups - fast pipeline fill,
    then big descriptors; 4KB-contig descriptors cap the HWDGE ring at
    ~258GB/s so later chunks use 2-8KB lines), x/h ahead of c; outputs in
    8/4/2/2-group chunks (big output DMAs fire early enough that their
    completion receipts stay off the critical tail) with per-group DMAs
    at the very end. ~9 warmup matmuls on a junk tile bridge the initial
    DMA wait so the PE's HAM activity window never resets during fill.
"""
import numpy as np
import ml_dtypes
from contextlib import ExitStack

import concourse.bass as bass
import concourse.tile as tile
from concourse import bacc, mybir
from concourse.bass_utils import run_bass_kernel_spmd

F32 = mybir.dt.float32
F16 = mybir.dt.float16
BF16 = mybir.dt.bfloat16
NPBF = ml_dtypes.bfloat16
AF = mybir.ActivationFunctionType
ALU = mybir.AluOpType

NCORES = 8
BC = 8192            # batch rows per core
GW = 512             # batch columns per group (one PSUM bank)
NG = BC // GW        # 16 groups
H = 128              # hidden size
# input chunks in groups: small (fast fill), then growing
ICHUNKS = [(0, 1), (1, 1), (2, 2), (4, 4), (8, 8)]
# output chunks (start group, n groups): big, medium, small tail
OCHUNKS = [(0, 8), (8, 4), (12, 2), (14, 2)]

_CACHE = {}


def _build(has_bias: bool):
    nc = bacc.Bacc("TRN2", target_bir_lowering=False, debug=False)
    xt = nc.dram_tensor("xt", [H, BC], F16, kind="ExternalInput").ap()
    ht = nc.dram_tensor("ht", [H, BC], F16, kind="ExternalInput").ap()
    ct = nc.dram_tensor("ct", [H, BC], F16, kind="ExternalInput").ap()
    wxt = nc.dram_tensor("wxt", [H, 4 * H], F16, kind="ExternalInput").ap()
    wht = nc.dram_tensor("wht", [H, 4 * H], F16, kind="ExternalInput").ap()
    if has_bias:
        bias = nc.dram_tensor("bias", [H, 4], F32, kind="ExternalInput").ap()
    hnt = nc.dram_tensor("hnt", [H, BC], F16, kind="ExternalOutput").ap()
    cnt = nc.dram_tensor("cnt", [H, BC], F16, kind="ExternalOutput").ap()



    with tile.TileContext(nc) as tc:
        with ExitStack() as ctx:
            const = ctx.enter_context(tc.tile_pool(name="const", bufs=1))
            ina = ctx.enter_context(tc.tile_pool(name="ina", bufs=1))
            qp = ctx.enter_context(tc.tile_pool(name="qp", bufs=2, space="PSUM"))
            tp = ctx.enter_context(tc.tile_pool(name="tp", bufs=2))
            sp = ctx.enter_context(tc.tile_pool(name="sp", bufs=6))
            op = ctx.enter_context(tc.tile_pool(name="op", bufs=2))

            # Input tiles in 3 chunks per tensor: small chunk first for fast
            # pipeline fill, then medium/large for DMA efficiency.  x/h
            # chunks issue before c (c is consumed later, by the DVE chain).
            xts, hts, cts = [], [], []
            for ci, (cs, cw) in enumerate(ICHUNKS):
                for lst, nm in ((xts, "x"), (hts, "h"), (cts, "c")):
                    lst.append(ina.tile([H, cw * GW], F16,
                                        name=f"{nm}{ci}"))
            def cbounds(ci):
                cs, cw = ICHUNKS[ci]
                return cs * GW, (cs + cw) * GW
            for ci in range(len(ICHUNKS)):
                c0, c1 = cbounds(ci)
                nc.sync.dma_start(xts[ci][:], xt[:, c0:c1])
                nc.sync.dma_start(hts[ci][:], ht[:, c0:c1])
                if ci == 0:
                    wx_sb = const.tile([H, 4 * H], F16)
                    nc.sync.dma_start(wx_sb[:], wxt)
                    wh_sb = const.tile([H, 4 * H], F16)
                    nc.sync.dma_start(wh_sb[:], wht)
                    if has_bias:
                        b_sb = const.tile([H, 4], F32)
                        nc.sync.dma_start(b_sb[:], bias)
                else:
                    c0p, c1p = cbounds(ci - 1)
                    nc.sync.dma_start(cts[ci - 1][:], ct[:, c0p:c1p])
            c0p, c1p = cbounds(len(ICHUNKS) - 1)
            nc.sync.dma_start(cts[-1][:], ct[:, c0p:c1p])

            # ACT table preload (sigmoid set includes tanh) overlaps DMA fill
            dummy = const.tile([H, 8], F32)
            nc.vector.memset(dummy[:], 0.0)
            dummy2 = const.tile([H, 8], F32)
            nc.scalar.activation(dummy2[:], dummy[:], AF.Sigmoid)

            def in_slice(tiles, g, w):
                c0 = g * GW
                for ci, (cs, cw) in enumerate(ICHUNKS):
                    if c0 + w <= (cs + cw) * GW:
                        return tiles[ci][:, c0 - cs * GW:c0 - cs * GW + w]
                raise AssertionError("slice straddles input chunks")

            # HAM warmup on a junk tile while DMAs stream
            junk = const.tile([H, GW], F16)
            nc.vector.memset(junk[:], 0.0)
            warm = qp.tile([H, 2048], F32, name="warm", tag="quad")
            for _ in range(9):
                nc.tensor.matmul(warm[:, 0:GW], junk[:, 0:H], junk[:],
                                 start=True, stop=True)

            NP = NG // 2  # pairs
            sig2s = {}

            # pair -> (chunk_start_group, chunk_width, local_offset, is_last)
            pair_chunk = {}
            for cs, cw in OCHUNKS:
                for g in range(cs, cs + cw, 2):
                    pair_chunk[g // 2] = (cs, cw * GW, (g - cs) * GW,
                                          g + 2 == cs + cw)

            def emit_tanh_h2(Pa):
                """tanh + h' for pairs (Pa, Pa+1) in one ACT pass."""
                Pb = Pa + 1
                cs, cw, lo_a, _ = pair_chunk[Pa]
                cnb, hnb = cn_hn[Pa]
                tcp = tp.tile([H, 2048], BF16, name=f"tc{Pa}", tag="tc")
                nc.scalar.activation(tcp[:], cnb[:, lo_a:lo_a + 4 * GW],
                                     AF.Tanh)
                for j, P in enumerate((Pa, Pb)):
                    lo = pair_chunk[P][2]
                    last = pair_chunk[P][3]
                    sig2 = sig2s.pop(P)
                    o3 = sig2[:].rearrange("p (t x) -> p t x",
                                           t=2)[:, :, 0:512]
                    h3 = hnb[:, lo:lo + 2 * GW].rearrange(
                        "p (t x) -> p t x", t=2)
                    t3 = tcp[:, j * 1024:(j + 1) * 1024].rearrange(
                        "p (t x) -> p t x", t=2)
                    nc.vector.tensor_mul(h3, o3, t3)
                    if last:
                        nc.sync.dma_start(hnt[:, cs * GW:cs * GW + cw],
                                          hnb[:])

            def emit_tanh_h(P):
                """tanh + h' + (maybe) hn DMA for pair P (c' already done)."""
                cs, cw, lo, last = pair_chunk[P]
                cnb, hnb = cn_hn[P]
                tcp = tp.tile([H, 1024], BF16, name=f"tc{P}", tag="tc")
                nc.scalar.activation(tcp[:], cnb[:, lo:lo + 2 * GW], AF.Tanh)
                sig2 = sig2s.pop(P)
                o3 = sig2[:].rearrange("p (t x) -> p t x", t=2)[:, :, 0:512]
                h3 = hnb[:, lo:lo + 2 * GW].rearrange("p (t x) -> p t x", t=2)
                t3 = tcp[:].rearrange("p (t x) -> p t x", t=2)
                nc.vector.tensor_mul(h3, o3, t3)
                if last:
                    nc.sync.dma_start(hnt[:, cs * GW:cs * GW + cw], hnb[:])

            cn_hn = {}
            cn_buf = hn_buf = None
            for P in range(NP):
                g0 = 2 * P
                cs, cw, lo, last = pair_chunk[P]
                if lo == 0:
                    cn_buf = op.tile([H, cw], F16, name=f"cn{g0}", tag="cn")
                    hn_buf = op.tile([H, cw], F16, name=f"hn{g0}", tag="hn")
                cn_hn[P] = (cn_buf, hn_buf)
                sig2 = sp.tile([H, 4096], BF16, name=f"s{P}", tag="sig")
                sig2s[P] = sig2

                def emit_dve(g_first, ng, tag_sfx):
                    """c'-chain for ng groups starting at g_first (pair P).
                    ig/fc/c' are fp16: bf16 rounding of the large ig/fc
                    terms would dominate the error after cancellation."""
                    w = ng * GW
                    gg = g_first - g0

                    def sl(bank):
                        s = sig2[:].rearrange("p (t x) -> p t x", t=2)
                        s = s[:, gg:gg + ng, bank * GW:(bank + 1) * GW]
                        return s

                    def r3(ap2d):
                        return ap2d.rearrange("p (t x) -> p t x", t=ng)

                    c3 = r3(in_slice(cts, g_first, w))
                    gt = tp.tile([H, w], BF16, name=f"gt{tag_sfx}", tag="gt")
                    nc.vector.tensor_scalar(r3(gt[:]), sl(0 + 3), 2.0, 1.0,
                                            ALU.mult, ALU.subtract)
                    ig = tp.tile([H, w], F16, name=f"ig{tag_sfx}", tag="ig")
                    nc.vector.tensor_mul(r3(ig[:]), sl(1), r3(gt[:]))
                    fc = tp.tile([H, w], F16, name=f"fc{tag_sfx}", tag="fc")
                    nc.vector.tensor_mul(r3(fc[:]), sl(2), c3)
                    lg = lo + gg * GW
                    nc.vector.tensor_add(cn_buf[:, lg:lg + w], ig[:], fc[:])
                    if last and gg + ng == 2:
                        nc.sync.dma_start(
                            cnt[:, cs * GW:cs * GW + cw], cn_buf[:])

                lastP = P == NP - 1
                for gg in range(2):
                    g = g0 + gg
                    xs = in_slice(xts, g, GW)
                    hs = in_slice(hts, g, GW)
                    split = (lastP or P == 0) and not has_bias
                    quad = qp.tile([H, 2048], F32, name=f"q{g}", tag="quad")
                    so = sig2[:, gg * 2048:(gg + 1) * 2048]
                    for k in ([1, 2, 3, 0] if split else range(4)):
                        nc.tensor.matmul(quad[:, k * GW:(k + 1) * GW],
                                         wx_sb[:, k * H:(k + 1) * H], xs,
                                         start=True, stop=False)
                        nc.tensor.matmul(quad[:, k * GW:(k + 1) * GW],
                                         wh_sb[:, k * H:(k + 1) * H], hs,
                                         start=False, stop=True)
                    if has_bias:
                        for k in range(4):
                            nc.scalar.activation(
                                so[:, k * GW:(k + 1) * GW],
                                quad[:, k * GW:(k + 1) * GW],
                                AF.Sigmoid, bias=b_sb[:, k:k + 1])
                    elif split:
                        # i/f/s banks first: unblocks the DVE chain; the
                        # o bank (only needed by h') trails
                        nc.scalar.activation(so[:, GW:], quad[:, GW:],
                                             AF.Sigmoid)
                        nc.scalar.activation(so[:, 0:GW], quad[:, 0:GW],
                                             AF.Sigmoid)
                    else:
                        nc.scalar.activation(so, quad[:], AF.Sigmoid)
                    if lastP or P == 0:
                        # per-group chain: shortens tail (last pair) and
                        # avoids straddling input chunks (first pair)
                        emit_dve(g, 1, f"p{P}g{gg}")
                    if gg == 1:
                        if P in (2, 4, 6):
                            emit_tanh_h2(P - 2)
                        elif P == 7:
                            emit_tanh_h(P - 1)

                if not (lastP or P == 0):
                    emit_dve(g0, 2, f"p{P}")

            # last pair: per-group tanh/h'/hn to shorten the kernel tail
            P = NP - 1
            cs, cw, lo, _ = pair_chunk[P]
            cnb, hnb = cn_hn[P]
            sig2 = sig2s.pop(P)
            for gg in range(2):
                lg = lo + gg * GW
                tcg = tp.tile([H, GW], BF16, name=f"tcz{gg}", tag="tc")
                nc.scalar.activation(tcg[:], cnb[:, lg:lg + GW], AF.Tanh)
                o2 = sig2[:, gg * 2048:gg * 2048 + 512]
                nc.vector.tensor_mul(hnb[:, lg:lg + GW], o2, tcg[:])
                gcol = (cs + gg * (cw // GW - 1)) * GW
                nc.sync.dma_start(hnt[:, gcol:gcol + GW],
                                  hnb[:, lg:lg + GW])
    nc.compile()
    return nc


def _run(inputs, trace=False, tmpdir=None):
    x = np.asarray(inputs["x"], dtype=np.float32)
    h = np.asarray(inputs["h_t"], dtype=np.float32)
    c = np.asarray(inputs["c_t"], dtype=np.float32)
    # gate order [i, f, o, g]; W_g/b_g scaled by 2 for the tanh-via-sigmoid
    wx = np.concatenate([inputs["W_io"], inputs["W_ii"], inputs["W_if"],
                         2.0 * np.asarray(inputs["W_ig"])], axis=0)
    wh = np.concatenate([inputs["W_ho"], inputs["W_hi"], inputs["W_hf"],
                         2.0 * np.asarray(inputs["W_hg"])], axis=0)
    b = np.concatenate([inputs["b_o"], inputs["b_i"], inputs["b_f"],
                        2.0 * np.asarray(inputs["b_g"])], axis=0)
    wxt = np.ascontiguousarray(wx.T).astype(np.float16)
    wht = np.ascontiguousarray(wh.T).astype(np.float16)
    has_bias = bool(np.any(b))

    key = has_bias
    if key not in _CACHE:
        _CACHE[key] = _build(has_bias)
    nc = _CACHE[key]

    x16 = x.astype(np.float16)
    h16 = h.astype(np.float16)
    c16 = c.astype(np.float16)
    in_maps = []
    for i in range(NCORES):
        s = slice(i * BC, (i + 1) * BC)
        m = {
            "xt": np.ascontiguousarray(x16[s].T),
            "ht": np.ascontiguousarray(h16[s].T),
            "ct": np.ascontiguousarray(c16[s].T),
            "wxt": wxt,
            "wht": wht,
        }
        if has_bias:
            m["bias"] = np.ascontiguousarray(
                b.reshape(4, H).T.astype(np.float32))
        in_maps.append(m)

    res = run_bass_kernel_spmd(nc, in_maps, core_ids=list(range(NCORES)),
                               trace=trace, tmpdir=tmpdir)
    h_new = np.empty((NCORES * BC, H), dtype=np.float32)
    c_new = np.empty((NCORES * BC, H), dtype=np.float32)
    for i, r in enumerate(res.results):
        s = slice(i * BC, (i + 1) * BC)
        h_new[s] = r["hnt"].T
        c_new[s] = r["cnt"].T
    return h_new, c_new, res


def kernel(**inputs):
    h_new, c_new, _ = _run(inputs, trace=False)
    return h_new, c_new


# revision 35
# speedup vs baseline: 1.0333x; 1.0333x over previous
"""LSTMCell on 8 Trainium2 NeuronCores, data-parallel over the batch.

Full inputs: x/h_t/c_t [65536,128] f32, 8 gate weight matrices [128,128],
4 biases [128]. Returns (h_new, c_new) as [65536,128] f32 each.

Design (v13, ~59.7us; fp16 matmul path, transposed layout, no on-device
transposes; steady state is ACT(sigmoid)-bound):
  - Host transposes x/h/c per core to [128 feat, 8192 batch] fp16 and
    pre-concats weights as WxT/WhT [128 in, 512 gate-rows] fp16 in gate
    order [o, i, f, 2*g] (g prescaled by 2 for the tanh-via-sigmoid trick;
    o first so the first/last pairs can sigmoid banks i|f|2g ahead of o,
    unblocking the DVE chain ~1us earlier at the fill and tail).
    fp16 (not bf16) operands: the bf16 rounding of x/h/W through the gates
    was the dominant error term (1.2e-2); fp16 cuts it ~8x at zero cost
    (PE streams fp16 == bf16: ~216-260ns issue period per N=512 matmul).
  - Per batch group of 512 cols: 8 matmuls (weights stationary) accumulate
    gates^T into a 4-bank PSUM quad [128, 2048] = o|i|f|2g pre-acts.
  - ONE sigmoid per quad -> bf16 SBUF (ACT 16-bit-out runs ~0.9ns/elem;
    bf16 out is fastest of the 16-bit options; f32-out would be 2x faster
    on ACT but forces the DVE chain to 1x mode = net loss). Two groups
    share a sig2 tile [128, 4096] so DVE ops batch per PAIR via 3D APs
    (2-byte dtypes keep the DVE 2x mode, ~0.67ns/elem).
  - DVE per pair: gt=2s-1 [TS], ig=i*gt, fc=f*c, c'=ig+fc, h'=o*tanh(c').
    ig/fc/c'/h' and the c input are fp16: bf16 rounding of the large ig/fc
    terms dominated the error after cancellation in c'.
  - tanh(c') on ACT, delayed one pair (emitted after the next pair's
    sigmoids) so ACT never stalls on the DVE chain; batched across 2 pairs
    mid-pipeline; per-group at the tail to shorten the critical path.
  - DMA: inputs in 5 chunks/tensor (1,1,2,4,8 groups - fast pipeline fill,
    then big descriptors; 4KB-contig descriptors cap the HWDGE ring at
    ~258GB/s so later chunks use 2-8KB lines), x/h ahead of c; outputs in
    8/4/2/2-group chunks (big output DMAs fire early enough that their
    completion receipts stay off the critical tail) with per-group DMAs
    at the very end. ~9 warmup matmuls on a junk tile bridge the initial
    DMA wait so the PE's HAM activity window never resets during fill.
"""
import numpy as np
import ml_dtypes
from contextlib import ExitStack

import concourse.bass as bass
import concourse.tile as tile
from concourse import bacc, mybir
from concourse.bass_utils import run_bass_kernel_spmd

F32 = mybir.dt.float32
F16 = mybir.dt.float16
BF16 = mybir.dt.bfloat16
NPBF = ml_dtypes.bfloat16
AF = mybir.ActivationFunctionType
ALU = mybir.AluOpType

NCORES = 8
BC = 8192            # batch rows per core
GW = 512             # batch columns per group (one PSUM bank)
NG = BC // GW        # 16 groups
H = 128              # hidden size
# input chunks in groups: small (fast fill), then growing
ICHUNKS = [(0, 1), (1, 1), (2, 2), (4, 4), (8, 8)]
# output chunks (start group, n groups): big, medium, small tail
OCHUNKS = [(0, 8), (8, 4), (12, 2), (14, 2)]

_CACHE = {}


def _build(has_bias: bool):
    nc = bacc.Bacc("TRN2", target_bir_lowering=False, debug=False)
    xt = nc.dram_tensor("xt", [H, BC], F16, kind="ExternalInput").ap()
    ht = nc.dram_tensor("ht", [H, BC], F16, kind="ExternalInput").ap()
    ct = nc.dram_tensor("ct", [H, BC], F16, kind="ExternalInput").ap()
    wxt = nc.dram_tensor("wxt", [H, 4 * H], F16, kind="ExternalInput").ap()
    wht = nc.dram_tensor("wht", [H, 4 * H], F16, kind="ExternalInput").ap()
    if has_bias:
        bias = nc.dram_tensor("bias", [H, 4], F32, kind="ExternalInput").ap()
    hnt = nc.dram_tensor("hnt", [H, BC], F16, kind="ExternalOutput").ap()
    cnt = nc.dram_tensor("cnt", [H, BC], F16, kind="ExternalOutput").ap()



    with tile.TileContext(nc) as tc:
        with ExitStack() as ctx:
            const = ctx.enter_context(tc.tile_pool(name="const", bufs=1))
            ina = ctx.enter_context(tc.tile_pool(name="ina", bufs=1))
            qp = ctx.enter_context(tc.tile_pool(name="qp", bufs=2, space="PSUM"))
            tp = ctx.enter_context(tc.tile_pool(name="tp", bufs=3))
            sp = ctx.enter_context(tc.tile_pool(name="sp", bufs=6))
            op = ctx.enter_context(tc.tile_pool(name="op", bufs=3))

            # Input tiles in 3 chunks per tensor: small chunk first for fast
            # pipeline fill, then medium/large for DMA efficiency.  x/h
            # chunks issue before c (c is consumed later, by the DVE chain).
            xts, hts, cts = [], [], []
            for ci, (cs, cw) in enumerate(ICHUNKS):
                for lst, nm in ((xts, "x"), (hts, "h"), (cts, "c")):
                    lst.append(ina.tile([H, cw * GW], F16,
                                        name=f"{nm}{ci}"))
            def cbounds(ci):
                cs, cw = ICHUNKS[ci]
                return cs * GW, (cs + cw) * GW
            for ci in range(len(ICHUNKS)):
                c0, c1 = cbounds(ci)
                nc.sync.dma_start(xts[ci][:], xt[:, c0:c1])
                nc.sync.dma_start(hts[ci][:], ht[:, c0:c1])
                if ci == 0:
                    wx_sb = const.tile([H, 4 * H], F16)
                    nc.sync.dma_start(wx_sb[:], wxt)
                    wh_sb = const.tile([H, 4 * H], F16)
                    nc.sync.dma_start(wh_sb[:], wht)
                    if has_bias:
                        b_sb = const.tile([H, 4], F32)
                        nc.sync.dma_start(b_sb[:], bias)
                else:
                    c0p, c1p = cbounds(ci - 1)
                    nc.sync.dma_start(cts[ci - 1][:], ct[:, c0p:c1p])
            c0p, c1p = cbounds(len(ICHUNKS) - 1)
            nc.sync.dma_start(cts[-1][:], ct[:, c0p:c1p])

            # ACT table preload (sigmoid set includes tanh) overlaps DMA fill
            dummy = const.tile([H, 8], F32)
            nc.vector.memset(dummy[:], 0.0)
            dummy2 = const.tile([H, 8], F32)
            nc.scalar.activation(dummy2[:], dummy[:], AF.Sigmoid)

            def in_slice(tiles, g, w):
                c0 = g * GW
                for ci, (cs, cw) in enumerate(ICHUNKS):
                    if c0 + w <= (cs + cw) * GW:
                        return tiles[ci][:, c0 - cs * GW:c0 - cs * GW + w]
                raise AssertionError("slice straddles input chunks")

            # HAM warmup on a junk tile while DMAs stream
            junk = const.tile([H, GW], F16)
            nc.vector.memset(junk[:], 0.0)
            warm = qp.tile([H, 2048], F32, name="warm", tag="quad")
            for _ in range(9):
                nc.tensor.matmul(warm[:, 0:GW], junk[:, 0:H], junk[:],
                                 start=True, stop=True)

            NP = NG // 2  # pairs
            sig2s = {}

            # pair -> (chunk_start_group, chunk_width, local_offset, is_last)
            pair_chunk = {}
            for cs, cw in OCHUNKS:
                for g in range(cs, cs + cw, 2):
                    pair_chunk[g // 2] = (cs, cw * GW, (g - cs) * GW,
                                          g + 2 == cs + cw)

            def emit_tanh_h2(Pa):
                """tanh + h' for pairs (Pa, Pa+1) in one ACT pass."""
                Pb = Pa + 1
                cs, cw, lo_a, _ = pair_chunk[Pa]
                cnb, hnb = cn_hn[Pa]
                tcp = tp.tile([H, 2048], BF16, name=f"tc{Pa}", tag="tc")
                nc.scalar.activation(tcp[:], cnb[:, lo_a:lo_a + 4 * GW],
                                     AF.Tanh)
                for j, P in enumerate((Pa, Pb)):
                    lo = pair_chunk[P][2]
                    last = pair_chunk[P][3]
                    sig2 = sig2s.pop(P)
                    o3 = sig2[:].rearrange("p (t x) -> p t x",
                                           t=2)[:, :, 0:512]
                    h3 = hnb[:, lo:lo + 2 * GW].rearrange(
                        "p (t x) -> p t x", t=2)
                    t3 = tcp[:, j * 1024:(j + 1) * 1024].rearrange(
                        "p (t x) -> p t x", t=2)
                    nc.vector.tensor_mul(h3, o3, t3)
                    if last:
                        nc.sync.dma_start(hnt[:, cs * GW:cs * GW + cw],
                                          hnb[:])

            def emit_tanh_h(P):
                """tanh + h' + (maybe) hn DMA for pair P (c' already done)."""
                cs, cw, lo, last = pair_chunk[P]
                cnb, hnb = cn_hn[P]
                tcp = tp.tile([H, 1024], BF16, name=f"tc{P}", tag="tc")
                nc.scalar.activation(tcp[:], cnb[:, lo:lo + 2 * GW], AF.Tanh)
                sig2 = sig2s.pop(P)
                o3 = sig2[:].rearrange("p (t x) -> p t x", t=2)[:, :, 0:512]
                h3 = hnb[:, lo:lo + 2 * GW].rearrange("p (t x) -> p t x", t=2)
                t3 = tcp[:].rearrange("p (t x) -> p t x", t=2)
                nc.vector.tensor_mul(h3, o3, t3)
                if last:
                    nc.sync.dma_start(hnt[:, cs * GW:cs * GW + cw], hnb[:])

            cn_hn = {}
            cn_buf = hn_buf = None
            for P in range(NP):
                g0 = 2 * P
                cs, cw, lo, last = pair_chunk[P]
                if lo == 0:
                    cn_buf = op.tile([H, cw], F16, name=f"cn{g0}", tag="cn")
                    hn_buf = op.tile([H, cw], F16, name=f"hn{g0}", tag="hn")
                cn_hn[P] = (cn_buf, hn_buf)
                sig2 = sp.tile([H, 4096], BF16, name=f"s{P}", tag="sig")
                sig2s[P] = sig2

                def emit_dve(g_first, ng, tag_sfx):
                    """c'-chain for ng groups starting at g_first (pair P).
                    ig/fc/c' are fp16: bf16 rounding of the large ig/fc
                    terms would dominate the error after cancellation."""
                    w = ng * GW
                    gg = g_first - g0

                    def sl(bank):
                        s = sig2[:].rearrange("p (t x) -> p t x", t=2)
                        s = s[:, gg:gg + ng, bank * GW:(bank + 1) * GW]
                        return s

                    def r3(ap2d):
                        return ap2d.rearrange("p (t x) -> p t x", t=ng)

                    c3 = r3(in_slice(cts, g_first, w))
                    gt = tp.tile([H, w], BF16, name=f"gt{tag_sfx}", tag="gt")
                    nc.vector.tensor_scalar(r3(gt[:]), sl(0 + 3), 2.0, 1.0,
                                            ALU.mult, ALU.subtract)
                    ig = tp.tile([H, w], F16, name=f"ig{tag_sfx}", tag="ig")
                    nc.vector.tensor_mul(r3(ig[:]), sl(1), r3(gt[:]))
                    fc = tp.tile([H, w], F16, name=f"fc{tag_sfx}", tag="fc")
                    nc.vector.tensor_mul(r3(fc[:]), sl(2), c3)
                    lg = lo + gg * GW
                    nc.vector.tensor_add(cn_buf[:, lg:lg + w], ig[:], fc[:])
                    if last and gg + ng == 2:
                        nc.sync.dma_start(
                            cnt[:, cs * GW:cs * GW + cw], cn_buf[:])

                lastP = P == NP - 1
                for gg in range(2):
                    g = g0 + gg
                    xs = in_slice(xts, g, GW)
                    hs = in_slice(hts, g, GW)
                    split = (lastP or P == 0) and not has_bias
                    quad = qp.tile([H, 2048], F32, name=f"q{g}", tag="quad")
                    so = sig2[:, gg * 2048:(gg + 1) * 2048]
                    for k in ([1, 2, 3, 0] if split else range(4)):
                        nc.tensor.matmul(quad[:, k * GW:(k + 1) * GW],
                                         wx_sb[:, k * H:(k + 1) * H], xs,
                                         start=True, stop=False)
                        nc.tensor.matmul(quad[:, k * GW:(k + 1) * GW],
                                         wh_sb[:, k * H:(k + 1) * H], hs,
                                         start=False, stop=True)
                    if has_bias:
                        for k in range(4):
                            nc.scalar.activation(
                                so[:, k * GW:(k + 1) * GW],
                                quad[:, k * GW:(k + 1) * GW],
                                AF.Sigmoid, bias=b_sb[:, k:k + 1])
                    elif split:
                        # i/f/s banks first: unblocks the DVE chain; the
                        # o bank (only needed by h') trails
                        nc.scalar.activation(so[:, GW:], quad[:, GW:],
                                             AF.Sigmoid)
                        nc.scalar.activation(so[:, 0:GW], quad[:, 0:GW],
                                             AF.Sigmoid)
                    else:
                        nc.scalar.activation(so, quad[:], AF.Sigmoid)
                    if lastP or P == 0:
                        # per-group chain: shortens tail (last pair) and
                        # avoids straddling input chunks (first pair)
                        emit_dve(g, 1, f"p{P}g{gg}")
                    if gg == 1:
                        if P in (2, 4, 6):
                            emit_tanh_h2(P - 2)
                        elif P == 7:
                            emit_tanh_h(P - 1)

                if not (lastP or P == 0):
                    emit_dve(g0, 2, f"p{P}")

            # last pair: per-group tanh/h'/hn to shorten the kernel tail
            P = NP - 1
            cs, cw, lo, _ = pair_chunk[P]
            cnb, hnb = cn_hn[P]
            sig2 = sig2s.pop(P)
            for gg in range(2):
                lg = lo + gg * GW
                tcg = tp.tile([H, GW], BF16, name=f"tcz{gg}", tag="tc")
                nc.scalar.activation(tcg[:], cnb[:, lg:lg + GW], AF.Tanh)
                o2 = sig2[:, gg * 2048:gg * 2048 + 512]
                nc.vector.tensor_mul(hnb[:, lg:lg + GW], o2, tcg[:])
                gcol = (cs + gg * (cw // GW - 1)) * GW
                nc.sync.dma_start(hnt[:, gcol:gcol + GW],
                                  hnb[:, lg:lg + GW])
    nc.compile()
    return nc


def _run(inputs, trace=False, tmpdir=None):
    x = np.asarray(inputs["x"], dtype=np.float32)
    h = np.asarray(inputs["h_t"], dtype=np.float32)
    c = np.asarray(inputs["c_t"], dtype=np.float32)
    # gate order [i, f, o, g]; W_g/b_g scaled by 2 for the tanh-via-sigmoid
    wx = np.concatenate([inputs["W_io"], inputs["W_ii"], inputs["W_if"],
                         2.0 * np.asarray(inputs["W_ig"])], axis=0)
    wh = np.concatenate([inputs["W_ho"], inputs["W_hi"], inputs["W_hf"],
                         2.0 * np.asarray(inputs["W_hg"])], axis=0)
    b = np.concatenate([inputs["b_o"], inputs["b_i"], inputs["b_f"],
                        2.0 * np.asarray(inputs["b_g"])], axis=0)
    wxt = np.ascontiguousarray(wx.T).astype(np.float16)
    wht = np.ascontiguousarray(wh.T).astype(np.float16)
    has_bias = bool(np.any(b))

    key = has_bias
    if key not in _CACHE:
        _CACHE[key] = _build(has_bias)
    nc = _CACHE[key]

    x16 = x.astype(np.float16)
    h16 = h.astype(np.float16)
    c16 = c.astype(np.float16)
    in_maps = []
    for i in range(NCORES):
        s = slice(i * BC, (i + 1) * BC)
        m = {
            "xt": np.ascontiguousarray(x16[s].T),
            "ht": np.ascontiguousarray(h16[s].T),
            "ct": np.ascontiguousarray(c16[s].T),
            "wxt": wxt,
            "wht": wht,
        }
        if has_bias:
            m["bias"] = np.ascontiguousarray(
                b.reshape(4, H).T.astype(np.float32))
        in_maps.append(m)

    res = run_bass_kernel_spmd(nc, in_maps, core_ids=list(range(NCORES)),
                               trace=trace, tmpdir=tmpdir)
    h_new = np.empty((NCORES * BC, H), dtype=np.float32)
    c_new = np.empty((NCORES * BC, H), dtype=np.float32)
    for i, r in enumerate(res.results):
        s = slice(i * BC, (i + 1) * BC)
        h_new[s] = r["hnt"].T
        c_new[s] = r["cnt"].T
    return h_new, c_new, res


def kernel(**inputs):
    h_new, c_new, _ = _run(inputs, trace=False)
    return h_new, c_new


# revision 36
# speedup vs baseline: 1.0579x; 1.0238x over previous
"""LSTMCell on 8 Trainium2 NeuronCores, data-parallel over the batch.

Full inputs: x/h_t/c_t [65536,128] f32, 8 gate weight matrices [128,128],
4 biases [128]. Returns (h_new, c_new) as [65536,128] f32 each.

Design (v13, ~59.7us; fp16 matmul path, transposed layout, no on-device
transposes; steady state is ACT(sigmoid)-bound):
  - Host transposes x/h/c per core to [128 feat, 8192 batch] fp16 and
    pre-concats weights as WxT/WhT [128 in, 512 gate-rows] fp16 in gate
    order [o, i, f, 2*g] (g prescaled by 2 for the tanh-via-sigmoid trick;
    o first so the first/last pairs can sigmoid banks i|f|2g ahead of o,
    unblocking the DVE chain ~1us earlier at the fill and tail).
    fp16 (not bf16) operands: the bf16 rounding of x/h/W through the gates
    was the dominant error term (1.2e-2); fp16 cuts it ~8x at zero cost
    (PE streams fp16 == bf16: ~216-260ns issue period per N=512 matmul).
  - Per batch group of 512 cols: 8 matmuls (weights stationary) accumulate
    gates^T into a 4-bank PSUM quad [128, 2048] = o|i|f|2g pre-acts.
  - ONE sigmoid per quad -> bf16 SBUF (ACT 16-bit-out runs ~0.9ns/elem;
    bf16 out is fastest of the 16-bit options; f32-out would be 2x faster
    on ACT but forces the DVE chain to 1x mode = net loss). Two groups
    share a sig2 tile [128, 4096] so DVE ops batch per PAIR via 3D APs
    (2-byte dtypes keep the DVE 2x mode, ~0.67ns/elem).
  - DVE per pair: gt=2s-1 [TS], ig=i*gt, fc=f*c, c'=ig+fc, h'=o*tanh(c').
    ig/fc/c'/h' and the c input are fp16: bf16 rounding of the large ig/fc
    terms dominated the error after cancellation in c'.
  - tanh(c') on ACT, delayed one pair (emitted after the next pair's
    sigmoids) so ACT never stalls on the DVE chain; batched across 2 pairs
    mid-pipeline; per-group at the tail to shorten the critical path.
  - DMA: inputs in 5 chunks/tensor (1,1,2,4,8 groups - fast pipeline fill,
    then big descriptors; 4KB-contig descriptors cap the HWDGE ring at
    ~258GB/s so later chunks use 2-8KB lines), x/h ahead of c; outputs in
    8/4/2/2-group chunks (big output DMAs fire early enough that their
    completion receipts stay off the critical tail) with per-group DMAs
    at the very end. ~9 warmup matmuls on a junk tile bridge the initial
    DMA wait so the PE's HAM activity window never resets during fill.
"""
import numpy as np
import ml_dtypes
from contextlib import ExitStack

import concourse.bass as bass
import concourse.tile as tile
from concourse import bacc, mybir
from concourse.bass_utils import run_bass_kernel_spmd

F32 = mybir.dt.float32
F16 = mybir.dt.float16
BF16 = mybir.dt.bfloat16
NPBF = ml_dtypes.bfloat16
AF = mybir.ActivationFunctionType
ALU = mybir.AluOpType

NCORES = 8
BC = 8192            # batch rows per core
GW = 512             # batch columns per group (one PSUM bank)
NG = BC // GW        # 16 groups
H = 128              # hidden size
# input chunks in groups: small (fast fill), then growing
ICHUNKS = [(0, 1), (1, 1), (2, 2), (4, 4), (8, 8)]
# output chunks (start group, n groups): big, medium, small tail
OCHUNKS = [(0, 8), (8, 4), (12, 2), (14, 2)]

_CACHE = {}


def _build(has_bias: bool):
    nc = bacc.Bacc("TRN2", target_bir_lowering=False, debug=False)
    xt = nc.dram_tensor("xt", [H, BC], F16, kind="ExternalInput").ap()
    ht = nc.dram_tensor("ht", [H, BC], F16, kind="ExternalInput").ap()
    ct = nc.dram_tensor("ct", [H, BC], F16, kind="ExternalInput").ap()
    wxt = nc.dram_tensor("wxt", [H, 4 * H], F16, kind="ExternalInput").ap()
    wht = nc.dram_tensor("wht", [H, 4 * H], F16, kind="ExternalInput").ap()
    if has_bias:
        bias = nc.dram_tensor("bias", [H, 4], F32, kind="ExternalInput").ap()
    hnt = nc.dram_tensor("hnt", [H, BC], F16, kind="ExternalOutput").ap()
    cnt = nc.dram_tensor("cnt", [H, BC], F16, kind="ExternalOutput").ap()



    with tile.TileContext(nc) as tc:
        with ExitStack() as ctx:
            const = ctx.enter_context(tc.tile_pool(name="const", bufs=1))
            ina = ctx.enter_context(tc.tile_pool(name="ina", bufs=1))
            qp = ctx.enter_context(tc.tile_pool(name="qp", bufs=2, space="PSUM"))
            tp = ctx.enter_context(tc.tile_pool(name="tp", bufs=3))
            sp = ctx.enter_context(tc.tile_pool(name="sp", bufs=6))
            op = ctx.enter_context(tc.tile_pool(name="op", bufs=3))

            # Input tiles in 3 chunks per tensor: small chunk first for fast
            # pipeline fill, then medium/large for DMA efficiency.  x/h
            # chunks issue before c (c is consumed later, by the DVE chain).
            xts, hts, cts = [], [], []
            for ci, (cs, cw) in enumerate(ICHUNKS):
                for lst, nm in ((xts, "x"), (hts, "h"), (cts, "c")):
                    lst.append(ina.tile([H, cw * GW], F16,
                                        name=f"{nm}{ci}"))
            def cbounds(ci):
                cs, cw = ICHUNKS[ci]
                return cs * GW, (cs + cw) * GW
            # x/h chunks 0-3 gate matmuls -> issue them first; c is only
            # consumed by the DVE chain (~7us of slack) so its chunks can
            # trail the x/h stream without stalling anything.
            nchunk = len(ICHUNKS)
            order = ([("xh", ci) for ci in range(nchunk - 1)] +
                     [("c", ci) for ci in range(3)] +
                     [("xh", nchunk - 1)] +
                     [("c", ci) for ci in range(3, nchunk)])
            for kind, ci in order:
                c0, c1 = cbounds(ci)
                if kind == "xh":
                    nc.sync.dma_start(xts[ci][:], xt[:, c0:c1])
                    nc.sync.dma_start(hts[ci][:], ht[:, c0:c1])
                    if ci == 0:
                        wx_sb = const.tile([H, 4 * H], F16)
                        nc.sync.dma_start(wx_sb[:], wxt)
                        wh_sb = const.tile([H, 4 * H], F16)
                        nc.sync.dma_start(wh_sb[:], wht)
                        if has_bias:
                            b_sb = const.tile([H, 4], F32)
                            nc.sync.dma_start(b_sb[:], bias)
                else:
                    nc.sync.dma_start(cts[ci][:], ct[:, c0:c1])

            # ACT table preload (sigmoid set includes tanh) overlaps DMA fill
            dummy = const.tile([H, 8], F32)
            nc.vector.memset(dummy[:], 0.0)
            dummy2 = const.tile([H, 8], F32)
            nc.scalar.activation(dummy2[:], dummy[:], AF.Sigmoid)

            def in_slice(tiles, g, w):
                c0 = g * GW
                for ci, (cs, cw) in enumerate(ICHUNKS):
                    if c0 + w <= (cs + cw) * GW:
                        return tiles[ci][:, c0 - cs * GW:c0 - cs * GW + w]
                raise AssertionError("slice straddles input chunks")

            # HAM warmup on a junk tile while DMAs stream
            junk = const.tile([H, GW], F16)
            nc.vector.memset(junk[:], 0.0)
            warm = qp.tile([H, 2048], F32, name="warm", tag="quad")
            for _ in range(9):
                nc.tensor.matmul(warm[:, 0:GW], junk[:, 0:H], junk[:],
                                 start=True, stop=True)

            NP = NG // 2  # pairs
            sig2s = {}

            # pair -> (chunk_start_group, chunk_width, local_offset, is_last)
            pair_chunk = {}
            for cs, cw in OCHUNKS:
                for g in range(cs, cs + cw, 2):
                    pair_chunk[g // 2] = (cs, cw * GW, (g - cs) * GW,
                                          g + 2 == cs + cw)

            def emit_tanh_h2(Pa):
                """tanh + h' for pairs (Pa, Pa+1) in one ACT pass."""
                Pb = Pa + 1
                cs, cw, lo_a, _ = pair_chunk[Pa]
                cnb, hnb = cn_hn[Pa]
                tcp = tp.tile([H, 2048], BF16, name=f"tc{Pa}", tag="tc")
                nc.scalar.activation(tcp[:], cnb[:, lo_a:lo_a + 4 * GW],
                                     AF.Tanh)
                for j, P in enumerate((Pa, Pb)):
                    lo = pair_chunk[P][2]
                    last = pair_chunk[P][3]
                    sig2 = sig2s.pop(P)
                    o3 = sig2[:].rearrange("p (t x) -> p t x",
                                           t=2)[:, :, 0:512]
                    h3 = hnb[:, lo:lo + 2 * GW].rearrange(
                        "p (t x) -> p t x", t=2)
                    t3 = tcp[:, j * 1024:(j + 1) * 1024].rearrange(
                        "p (t x) -> p t x", t=2)
                    nc.vector.tensor_mul(h3, o3, t3)
                    if last:
                        nc.sync.dma_start(hnt[:, cs * GW:cs * GW + cw],
                                          hnb[:])

            def emit_tanh_h(P):
                """tanh + h' + (maybe) hn DMA for pair P (c' already done)."""
                cs, cw, lo, last = pair_chunk[P]
                cnb, hnb = cn_hn[P]
                tcp = tp.tile([H, 1024], BF16, name=f"tc{P}", tag="tc")
                nc.scalar.activation(tcp[:], cnb[:, lo:lo + 2 * GW], AF.Tanh)
                sig2 = sig2s.pop(P)
                o3 = sig2[:].rearrange("p (t x) -> p t x", t=2)[:, :, 0:512]
                h3 = hnb[:, lo:lo + 2 * GW].rearrange("p (t x) -> p t x", t=2)
                t3 = tcp[:].rearrange("p (t x) -> p t x", t=2)
                nc.vector.tensor_mul(h3, o3, t3)
                if last:
                    nc.sync.dma_start(hnt[:, cs * GW:cs * GW + cw], hnb[:])

            cn_hn = {}
            cn_buf = hn_buf = None
            for P in range(NP):
                g0 = 2 * P
                cs, cw, lo, last = pair_chunk[P]
                if lo == 0:
                    cn_buf = op.tile([H, cw], F16, name=f"cn{g0}", tag="cn")
                    hn_buf = op.tile([H, cw], F16, name=f"hn{g0}", tag="hn")
                cn_hn[P] = (cn_buf, hn_buf)
                sig2 = sp.tile([H, 4096], BF16, name=f"s{P}", tag="sig")
                sig2s[P] = sig2

                def emit_dve(g_first, ng, tag_sfx):
                    """c'-chain for ng groups starting at g_first (pair P).
                    ig/fc/c' are fp16: bf16 rounding of the large ig/fc
                    terms would dominate the error after cancellation."""
                    w = ng * GW
                    gg = g_first - g0

                    def sl(bank):
                        s = sig2[:].rearrange("p (t x) -> p t x", t=2)
                        s = s[:, gg:gg + ng, bank * GW:(bank + 1) * GW]
                        return s

                    def r3(ap2d):
                        return ap2d.rearrange("p (t x) -> p t x", t=ng)

                    c3 = r3(in_slice(cts, g_first, w))
                    gt = tp.tile([H, w], BF16, name=f"gt{tag_sfx}", tag="gt")
                    nc.vector.tensor_scalar(r3(gt[:]), sl(0 + 3), 2.0, 1.0,
                                            ALU.mult, ALU.subtract)
                    ig = tp.tile([H, w], F16, name=f"ig{tag_sfx}", tag="ig")
                    nc.vector.tensor_mul(r3(ig[:]), sl(1), r3(gt[:]))
                    fc = tp.tile([H, w], F16, name=f"fc{tag_sfx}", tag="fc")
                    nc.vector.tensor_mul(r3(fc[:]), sl(2), c3)
                    lg = lo + gg * GW
                    nc.vector.tensor_add(cn_buf[:, lg:lg + w], ig[:], fc[:])
                    if last and gg + ng == 2:
                        nc.sync.dma_start(
                            cnt[:, cs * GW:cs * GW + cw], cn_buf[:])

                lastP = P == NP - 1
                for gg in range(2):
                    g = g0 + gg
                    xs = in_slice(xts, g, GW)
                    hs = in_slice(hts, g, GW)
                    split = (lastP or P == 0) and not has_bias
                    quad = qp.tile([H, 2048], F32, name=f"q{g}", tag="quad")
                    so = sig2[:, gg * 2048:(gg + 1) * 2048]
                    for k in ([1, 2, 3, 0] if split else range(4)):
                        nc.tensor.matmul(quad[:, k * GW:(k + 1) * GW],
                                         wx_sb[:, k * H:(k + 1) * H], xs,
                                         start=True, stop=False)
                        nc.tensor.matmul(quad[:, k * GW:(k + 1) * GW],
                                         wh_sb[:, k * H:(k + 1) * H], hs,
                                         start=False, stop=True)
                    if has_bias:
                        for k in range(4):
                            nc.scalar.activation(
                                so[:, k * GW:(k + 1) * GW],
                                quad[:, k * GW:(k + 1) * GW],
                                AF.Sigmoid, bias=b_sb[:, k:k + 1])
                    elif split:
                        # i/f/s banks first: unblocks the DVE chain; the
                        # o bank (only needed by h') trails
                        nc.scalar.activation(so[:, GW:], quad[:, GW:],
                                             AF.Sigmoid)
                        nc.scalar.activation(so[:, 0:GW], quad[:, 0:GW],
                                             AF.Sigmoid)
                    else:
                        nc.scalar.activation(so, quad[:], AF.Sigmoid)
                    if lastP or P == 0:
                        # per-group chain: shortens tail (last pair) and
                        # avoids straddling input chunks (first pair)
                        emit_dve(g, 1, f"p{P}g{gg}")
                    if gg == 1:
                        if P in (2, 4, 6):
                            emit_tanh_h2(P - 2)
                        elif P == 7:
                            emit_tanh_h(P - 1)

                if not (lastP or P == 0):
                    emit_dve(g0, 2, f"p{P}")

            # last pair: per-group tanh/h'/hn to shorten the kernel tail
            P = NP - 1
            cs, cw, lo, _ = pair_chunk[P]
            cnb, hnb = cn_hn[P]
            sig2 = sig2s.pop(P)
            for gg in range(2):
                lg = lo + gg * GW
                tcg = tp.tile([H, GW], BF16, name=f"tcz{gg}", tag="tc")
                nc.scalar.activation(tcg[:], cnb[:, lg:lg + GW], AF.Tanh)
                o2 = sig2[:, gg * 2048:gg * 2048 + 512]
                nc.vector.tensor_mul(hnb[:, lg:lg + GW], o2, tcg[:])
                gcol = (cs + gg * (cw // GW - 1)) * GW
                nc.sync.dma_start(hnt[:, gcol:gcol + GW],
                                  hnb[:, lg:lg + GW])
    nc.compile()
    return nc


def _run(inputs, trace=False, tmpdir=None):
    x = np.asarray(inputs["x"], dtype=np.float32)
    h = np.asarray(inputs["h_t"], dtype=np.float32)
    c = np.asarray(inputs["c_t"], dtype=np.float32)
    # gate order [i, f, o, g]; W_g/b_g scaled by 2 for the tanh-via-sigmoid
    wx = np.concatenate([inputs["W_io"], inputs["W_ii"], inputs["W_if"],
                         2.0 * np.asarray(inputs["W_ig"])], axis=0)
    wh = np.concatenate([inputs["W_ho"], inputs["W_hi"], inputs["W_hf"],
                         2.0 * np.asarray(inputs["W_hg"])], axis=0)
    b = np.concatenate([inputs["b_o"], inputs["b_i"], inputs["b_f"],
                        2.0 * np.asarray(inputs["b_g"])], axis=0)
    wxt = np.ascontiguousarray(wx.T).astype(np.float16)
    wht = np.ascontiguousarray(wh.T).astype(np.float16)
    has_bias = bool(np.any(b))

    key = has_bias
    if key not in _CACHE:
        _CACHE[key] = _build(has_bias)
    nc = _CACHE[key]

    x16 = x.astype(np.float16)
    h16 = h.astype(np.float16)
    c16 = c.astype(np.float16)
    in_maps = []
    for i in range(NCORES):
        s = slice(i * BC, (i + 1) * BC)
        m = {
            "xt": np.ascontiguousarray(x16[s].T),
            "ht": np.ascontiguousarray(h16[s].T),
            "ct": np.ascontiguousarray(c16[s].T),
            "wxt": wxt,
            "wht": wht,
        }
        if has_bias:
            m["bias"] = np.ascontiguousarray(
                b.reshape(4, H).T.astype(np.float32))
        in_maps.append(m)

    res = run_bass_kernel_spmd(nc, in_maps, core_ids=list(range(NCORES)),
                               trace=trace, tmpdir=tmpdir)
    h_new = np.empty((NCORES * BC, H), dtype=np.float32)
    c_new = np.empty((NCORES * BC, H), dtype=np.float32)
    for i, r in enumerate(res.results):
        s = slice(i * BC, (i + 1) * BC)
        h_new[s] = r["hnt"].T
        c_new[s] = r["cnt"].T
    return h_new, c_new, res


def kernel(**inputs):
    h_new, c_new, _ = _run(inputs, trace=False)
    return h_new, c_new
